# revision 4
# baseline (speedup 1.0000x reference)
"""AttentionBlock Trainium2 kernel (self-contained).

Problem: x[4,256,64,64] -> qkv 1x1 conv -> single-head attention over the
4096 spatial tokens -> out 1x1 conv -> residual.

Sharding: 8 cores = 4 batch elements x 2 query halves. Each core handles one
batch element's full K/V (4096 tokens) and 2048 queries, flash-style on-chip
(scores never touch HBM).

Per-core dataflow (feature-major x = x[b] reshaped [256, 4096]):
  - qkv projections on PE: k,q feature-major [e, tok]; v token-major
    [tok, e] with a ones-column appended (softmax normalizer comes out of
    the attn@v matmul for free).
  - scores computed k-major: S^T[k, q] = k_fm.T-contraction - this makes
    exp(S^T) directly usable as the stationary operand of attn@v with no
    transposes of the big [4096 x 2048] attention matrix.
  - softmax without max-subtraction (scores are O(+-6.2) for this data;
    exp stays in [2e-3, 470], exact in fp32).
  - attn@v gives o token-major [q, e] plus Z in column 256; normalize by
    1/Z per-partition, PE-transpose 128x128 blocks to feature-major,
    out-projection, +bias, +residual, DMA out.

Precision: scores/projections in float32r (tf32-like, full PE rate at
free-dim>=256, ~1.6e-4 rel err), exp(S) and v in bf16 for the attn@v
matmul (errors there are diluted ~10x by the residual-dominated output).
"""

import numpy as np

import concourse.bass as bass
import concourse.bacc as bacc
import concourse.tile as tile
from concourse import mybir
from concourse.bass_utils import run_bass_kernel_spmd

F32 = mybir.dt.float32
F32R = mybir.dt.float32r
BF16 = mybir.dt.bfloat16
AF = mybir.ActivationFunctionType

E = 256          # embed dim
NTOK = 4096      # tokens per batch element (64*64)
NQ = 2048        # queries per core
P = 128          # partitions
NEC = 2          # e-chunks (E / P)
NKC = NTOK // P  # 32 k-chunks
QB = 256         # q block (scores free dim)
NQB = NQ // QB   # 8 q blocks
EXP_SCALE = 1.0 / 16.0  # 1/sqrt(E)

N_CORES = 8


def r(ap):
    return ap.bitcast(F32R)


def build_nc():
    nc = bacc.Bacc(None, target_bir_lowering=False)

    xb = nc.dram_tensor("xb", [E, NTOK], F32R, kind="ExternalInput")
    xq = nc.dram_tensor("xq", [E, NQ], F32R, kind="ExternalInput")
    wqkvT = nc.dram_tensor("wqkvT", [E, 3 * E], F32R, kind="ExternalInput")
    bqk = nc.dram_tensor("bqk", [P, 4], F32, kind="ExternalInput")
    bv = nc.dram_tensor("bv", [E], F32, kind="ExternalInput")
    woT = nc.dram_tensor("woT", [E, E], F32R, kind="ExternalInput")
    outb = nc.dram_tensor("outb", [P, 2], F32, kind="ExternalInput")
    ident = nc.dram_tensor("ident", [P, P], F32R, kind="ExternalInput")
    out = nc.dram_tensor("out", [E, NQ], F32, kind="ExternalOutput")

    with tile.TileContext(nc) as tc:
        with (
            tc.tile_pool(name="const", bufs=1) as const,
            tc.tile_pool(name="xpool", bufs=1) as xpool,
            tc.tile_pool(name="kqv", bufs=1) as kqv,
            tc.tile_pool(name="expp", bufs=2) as expp,
            tc.tile_pool(name="ofm", bufs=1) as ofm,
            tc.tile_pool(name="small", bufs=4) as small,
            tc.tile_pool(name="outp", bufs=3) as outp,
            tc.tile_pool(name="psA", bufs=2, space="PSUM") as psA,
            tc.tile_pool(name="psO", bufs=2, space="PSUM") as psO,
            tc.tile_pool(name="psT", bufs=2, space="PSUM") as psT,
        ):
            # ---- constants
            wqkvT_sb = const.tile([P, NEC, 3 * E], F32R, tag="wqkvT")
            for ec in range(NEC):
                nc.sync.dma_start(out=wqkvT_sb[:, ec, :],
                                  in_=wqkvT[ec * P:(ec + 1) * P, :])
            woT_sb = const.tile([P, NEC, E], F32R, tag="woT")
            for ec in range(NEC):
                nc.sync.dma_start(out=woT_sb[:, ec, :],
                                  in_=woT[ec * P:(ec + 1) * P, :])
            bqk_sb = const.tile([P, 4], F32, tag="bqk")
            nc.sync.dma_start(out=bqk_sb, in_=bqk[:, :])
            outb_sb = const.tile([P, 2], F32, tag="outb")
            nc.sync.dma_start(out=outb_sb, in_=outb[:, :])
            bv_bc = const.tile([P, E], F32, tag="bv")
            nc.sync.dma_start(out=bv_bc, in_=bv[:].partition_broadcast(P))
            ident_sb = const.tile([P, P], F32R, tag="ident")
            nc.sync.dma_start(out=ident_sb, in_=ident[:, :])

            # ---- x loads
            xb_sb = xpool.tile([P, NEC, NTOK], F32R, tag="xb")
            for ec in range(NEC):
                for tt in range(8):
                    nc.sync.dma_start(
                        out=xb_sb[:, ec, tt * 512:(tt + 1) * 512],
                        in_=xb[ec * P:(ec + 1) * P, tt * 512:(tt + 1) * 512])
            xq_sb = xpool.tile([P, NEC, NQ], F32R, tag="xq")
            for ec in range(NEC):
                nc.sync.dma_start(out=xq_sb[:, ec, :],
                                  in_=xq[ec * P:(ec + 1) * P, :])

            k_sb = kqv.tile([P, NEC, NTOK], F32R, tag="k")
            q_sb = kqv.tile([P, NEC, NQ], F32R, tag="q")
            v_sb = kqv.tile([P, NKC, E + 1], BF16, tag="v")

            # ---- K projection (feature-major): k[f, t] for f in 256..511
            for fc in range(NEC):
                for tt in range(8):
                    ps = psA.tile([P, 512], F32, tag="proj")
                    for ec in range(NEC):
                        nc.tensor.matmul(
                            ps,
                            (wqkvT_sb[:, ec, E + fc * P:E + (fc + 1) * P]),
                            (xb_sb[:, ec, tt * 512:(tt + 1) * 512]),
                            start=(ec == 0), stop=(ec == NEC - 1))
                    nc.vector.tensor_scalar_add(
                        k_sb[:, fc, tt * 512:(tt + 1) * 512], ps,
                        bqk_sb[:, 2 + fc:3 + fc])

            # ---- Q projection (feature-major, q-slice tokens)
            for fc in range(NEC):
                for tt in range(NQ // 512):
                    ps = psA.tile([P, 512], F32, tag="proj")
                    for ec in range(NEC):
                        nc.tensor.matmul(
                            ps,
                            (wqkvT_sb[:, ec, fc * P:(fc + 1) * P]),
                            (xq_sb[:, ec, tt * 512:(tt + 1) * 512]),
                            start=(ec == 0), stop=(ec == NEC - 1))
                    nc.vector.tensor_scalar_add(
                        q_sb[:, fc, tt * 512:(tt + 1) * 512], ps,
                        bqk_sb[:, fc:fc + 1])

            # ---- V projection (token-major): v[t, f] for f in 512..767
            for tcb in range(NKC):
                ps = psA.tile([P, E], F32, tag="sc")
                for ec in range(NEC):
                    nc.tensor.matmul(
                        ps,
                        (xb_sb[:, ec, tcb * P:(tcb + 1) * P]),
                        (wqkvT_sb[:, ec, 2 * E:3 * E]),
                        start=(ec == 0), stop=(ec == NEC - 1))
                nc.vector.tensor_add(v_sb[:, tcb, 0:E], ps, bv_bc)
            nc.vector.memset(v_sb[:, :, E:E + 1], 1.0)

            o_fm = ofm.tile([P, NEC, NQ], F32R, tag="o_fm")

            # ---- attention: per q block of QB queries
            for qb in range(NQB):
                q0 = qb * QB
                expS = expp.tile([P, NKC, QB], BF16, tag="expS")
                for kc in range(NKC):
                    ps = psA.tile([P, QB], F32, tag="sc")
                    for ec in range(NEC):
                        nc.tensor.matmul(
                            ps,
                            (k_sb[:, ec, kc * P:(kc + 1) * P]),
                            (q_sb[:, ec, q0:q0 + QB]),
                            start=(ec == 0), stop=(ec == NEC - 1))
                    nc.scalar.activation(expS[:, kc, :], ps, AF.Exp,
                                         scale=EXP_SCALE)
                for qq in range(QB // P):
                    po = psO.tile([P, E + 1], F32, tag="po")
                    for kc in range(NKC):
                        nc.tensor.matmul(
                            po,
                            expS[:, kc, qq * P:(qq + 1) * P],
                            v_sb[:, kc, :],
                            start=(kc == 0), stop=(kc == NKC - 1))
                    zr = small.tile([P, 1], F32, tag="zr")
                    nc.vector.reciprocal(zr, po[:, E:E + 1])
                    o_tm = small.tile([P, E], F32R, tag="o_tm")
                    nc.vector.tensor_scalar_mul(o_tm, po[:, 0:E], zr)
                    for ec in range(NEC):
                        pt = psT.tile([P, P], F32R, tag="pt")
                        nc.tensor.transpose(
                            pt, o_tm[:, ec * P:(ec + 1) * P], ident_sb)
                        nc.vector.tensor_copy(
                            o_fm[:, ec, q0 + qq * P:q0 + (qq + 1) * P], pt)

            # ---- out projection + bias + residual
            for qb in range(NQB):
                q0 = qb * QB
                for fc in range(NEC):
                    ps = psA.tile([P, QB], F32, tag="sc")
                    for ec in range(NEC):
                        nc.tensor.matmul(
                            ps,
                            (woT_sb[:, ec, fc * P:(fc + 1) * P]),
                            (o_fm[:, ec, q0:q0 + QB]),
                            start=(ec == 0), stop=(ec == NEC - 1))
                    t1 = outp.tile([P, QB], F32, tag="t1")
                    nc.scalar.activation(t1, ps, AF.Identity,
                                         bias=outb_sb[:, fc:fc + 1])
                    t2 = outp.tile([P, QB], F32, tag="t2")
                    nc.vector.tensor_add(t2, t1, xq_sb[:, fc, q0:q0 + QB].bitcast(F32))
                    nc.sync.dma_start(
                        out=out[fc * P:(fc + 1) * P, q0:q0 + QB], in_=t2)

    nc.compile()
    return nc


_NC = None


def _get_nc():
    global _NC
    if _NC is None:
        _NC = build_nc()
    return _NC


def make_in_maps(x, qkv_w, qkv_b, out_w, out_b):
    b, e, h, w = x.shape
    n = h * w
    xf = np.ascontiguousarray(x.reshape(b, e, n).astype(np.float32))
    wqkvT = np.ascontiguousarray(qkv_w.astype(np.float32).T)
    woT = np.ascontiguousarray(out_w.astype(np.float32).T)
    bqk_a = np.ascontiguousarray(
        qkv_b[:2 * E].astype(np.float32).reshape(4, P).T)
    bv_a = np.ascontiguousarray(qkv_b[2 * E:].astype(np.float32))
    outb_a = np.ascontiguousarray(out_b.astype(np.float32).reshape(2, P).T)
    ident = np.eye(P, dtype=np.float32)
    in_maps = []
    for core in range(N_CORES):
        bi, half = divmod(core, 2)
        sl = slice(half * NQ, (half + 1) * NQ)
        in_maps.append({
            "xb": xf[bi],
            "xq": np.ascontiguousarray(xf[bi][:, sl]),
            "wqkvT": wqkvT, "bqk": bqk_a, "bv": bv_a,
            "woT": woT, "outb": outb_a, "ident": ident,
        })
    return in_maps


def assemble(results, x_shape):
    b, e, h, w = x_shape
    n = h * w
    out = np.empty((b, e, n), np.float32)
    for core in range(N_CORES):
        bi, half = divmod(core, 2)
        out[bi][:, half * NQ:(half + 1) * NQ] = results[core]["out"]
    return out.reshape(b, e, h, w)


def kernel(x, qkv_w, qkv_b, out_w, out_b):
    in_maps = make_in_maps(x, qkv_w, qkv_b, out_w, out_b)
    nc = _get_nc()
    res = run_bass_kernel_spmd(nc, in_maps, core_ids=list(range(N_CORES)))
    return assemble(res.results, x.shape)
